# revision 11
# baseline (speedup 1.0000x reference)
"""nn_ComplexNetAttention on 8 trn2 NeuronCores.

Sharding: heads column-parallel for QKV+attention (2 heads/core),
per-head f16 AllToAll to redistribute attention output to
token-sharded layout, token-parallel o-projection (256 tokens/core).

Activations are shipped host-side fake-quantized (round(x*s)/s) in
f16 so every projection runs as long same-PSUM-bank accumulation
chains with no vector dependency (keeps the PE array at its ramped
2.4 GHz p-state). Attention e-tiles are bf16 (fp32 exponent range -
no exp overflow). O-projection uses the Karatsuba 3-multiply complex
product on re-quantized+dequantized f16 activations.
"""
import numpy as np
import ml_dtypes

import concourse.bass as bass
import concourse.bacc as bacc
import concourse.tile as tile
import concourse.mybir as mybir
from concourse.bass_utils import run_bass_kernel_spmd

f32 = mybir.dt.float32
f16 = mybir.dt.float16
bf16 = mybir.dt.bfloat16

T, H, NH, D = 2048, 2048, 16, 128
NC = 8
HPC = NH // NC          # heads per core = 2
DS = HPC * D            # d_out slice per core = 256
TS = T // NC            # tokens per core for o-proj = 256
HT = H // 128           # 16 contraction chunks
NG = 4                  # token groups of 512
GW = T // NG            # 512
ROWS1 = 2 * 128         # A2A#1 rows/src: h0_r | h0_i
ROWS2 = 2 * 128 + 4     # A2A#2 rows/src: h1_r | h1_i | maxr_hi/lo | maxi_hi/lo
MAGIC = float(2**23 + 2**22)
A_OP = mybir.AluOpType


def build_nc():
    nc = bacc.Bacc("TRN2", target_bir_lowering=False, debug=False, num_devices=NC)
    A = {}
    def inp(name, shape, dt=f16):
        A[name] = nc.dram_tensor(name, shape, dt, kind="ExternalInput").ap()
    inp("a_r", [H, T]); inp("a_i", [H, T])
    inp("cosT", [D, T]); inp("sinT", [D, T])
    for tn in ("q", "k"):
        inp(f"w{tn}_r", [H, DS]); inp(f"w{tn}_i", [H, DS]); inp(f"w{tn}_rn", [H, DS])
    inp("wv1", [H, 2 * DS]); inp("wv2", [H, 2 * DS])
    inp("wo_i", [H, H]); inp("wo_m", [H, H]); inp("wo_p", [H, H])
    inp("masks", [128, 4 * 512])
    inp("ident", [128, 128], f32)
    A["yr_part"] = nc.dram_tensor("yr_part", [TS, H], f32, kind="ExternalOutput").ap()
    A["yi_part"] = nc.dram_tensor("yi_part", [TS, H], f32, kind="ExternalOutput").ap()
    return nc, A


def _chunked(ap):
    """DRAM [H, w] -> [128, HT, w] view (partition, h-chunk, col)."""
    return ap.rearrange("(a b) c -> b a c", b=128)


def emit(nc, A, tc, ctx):
    const = ctx.enter_context(tc.tile_pool(name="const", bufs=1))
    ps = ctx.enter_context(tc.tile_pool(name="ps", bufs=1, space="PSUM"))
    dram = ctx.enter_context(tc.tile_pool(name="dram", bufs=1, space="DRAM"))

    ident = const.tile([128, 128], f32, name="ident_t")
    nc.sync.dma_start(ident[:], A["ident"][:])

    _pn = [0]
    def psum(tag, shape=(128, 512)):
        _pn[0] += 1
        return ps.tile(list(shape), f32, name=f"pt{_pn[0]}", tag=tag)

    cont1 = dram.tile([NC * ROWS1, TS], f16, name="cont1")
    ag1 = dram.tile([NC * ROWS1, TS], f16, name="ag1")
    cont2 = dram.tile([NC * ROWS2, TS], f16, name="cont2")
    ag2 = dram.tile([NC * ROWS2, TS], f16, name="ag2")

    achk_r = _chunked(A["a_r"][:])
    achk_i = _chunked(A["a_i"][:])

    qk_ctx = tc.tile_pool(name="qk", bufs=1)
    vj_ctx = tc.tile_pool(name="vj", bufs=1)
    qk_pool = qk_ctx.__enter__()
    vj_pool = vj_ctx.__enter__()
    qrot = {}
    for tn in ("q", "k"):
        for hd in range(HPC):
            for cp in ("r", "i"):
                qrot[(tn, hd, cp)] = qk_pool.tile([128, T], bf16, name=f"{tn}rot{hd}{cp}")
    vjoin = {}
    for hd in range(HPC):
        for bk in range(T // 128):
            vjoin[(hd, bk)] = vj_pool.tile([128, 257], bf16, name=f"vj{hd}_{bk}")
            nc.vector.memset(vjoin[(hd, bk)][:, 256:257], 1.0)

    # ======== phase A: Q/K/V projections + rope, one acts pass ========
    with tc.tile_pool(name="pw", bufs=1) as pw, \
         tc.tile_pool(name="acts", bufs=2) as acts_p, \
         tc.tile_pool(name="tmp", bufs=2) as tmp:
        w = {}
        for kind in ("r", "i", "rn"):
            w[("q", kind)] = pw.tile([128, HT, DS], f16, name=f"wq_{kind}")
            nc.sync.dma_start(w[("q", kind)][:], _chunked(A[f"wq_{kind}"][:]))
        acts0 = {}
        for nm, src_ in (("ar", achk_r), ("ai", achk_i)):
            t = acts_p.tile([128, HT, GW], f16, name=f"{nm}0", tag=nm)
            nc.sync.dma_start(t[:], src_[:, :, 0:GW])
            acts0[nm] = t
        for kind in ("r", "i", "rn"):
            w[("k", kind)] = pw.tile([128, HT, DS], f16, name=f"wk_{kind}")
            nc.sync.dma_start(w[("k", kind)][:], _chunked(A[f"wk_{kind}"][:]))
        cosT = pw.tile([D, T], f16, name="cosT_t")
        nc.sync.dma_start(cosT[:], A["cosT"][:])
        sinT = pw.tile([D, T], f16, name="sinT_t")
        nc.sync.dma_start(sinT[:], A["sinT"][:])
        wv1 = pw.tile([128, HT, 2 * DS], f16, name="wv1_t")
        nc.sync.dma_start(wv1[:], _chunked(A["wv1"][:]))
        wv2 = pw.tile([128, HT, 2 * DS], f16, name="wv2_t")
        nc.sync.dma_start(wv2[:], _chunked(A["wv2"][:]))

        def rope_drain(tn, dt_, gsl, pA, pB):
            t1 = tmp.tile([128, GW], f16, name=f"t1{tn}{dt_}{gsl.start}", tag="t1")
            t2 = tmp.tile([128, GW], f16, name=f"t2{tn}{dt_}{gsl.start}", tag="t2")
            nc.vector.tensor_tensor(t1[:], pA[:], cosT[:, gsl], A_OP.mult)
            nc.vector.tensor_tensor(t2[:], pB[:], sinT[:, gsl], A_OP.mult)
            nc.vector.tensor_tensor(qrot[(tn, dt_, "r")][:, gsl], t1[:], t2[:], A_OP.subtract)
            t3 = tmp.tile([128, GW], f16, name=f"t3{tn}{dt_}{gsl.start}", tag="t1")
            t4 = tmp.tile([128, GW], f16, name=f"t4{tn}{dt_}{gsl.start}", tag="t2")
            nc.vector.tensor_tensor(t3[:], pB[:], cosT[:, gsl], A_OP.mult)
            nc.vector.tensor_tensor(t4[:], pA[:], sinT[:, gsl], A_OP.mult)
            nc.vector.tensor_tensor(qrot[(tn, dt_, "i")][:, gsl], t3[:], t4[:], A_OP.add)

        for g in range(NG):
            gsl = slice(g * GW, (g + 1) * GW)
            if g == 0:
                ar, ai = acts0["ar"], acts0["ai"]
            else:
                ar = acts_p.tile([128, HT, GW], f16, name=f"ar{g}", tag="ar")
                ai = acts_p.tile([128, HT, GW], f16, name=f"ai{g}", tag="ai")
                nc.sync.dma_start(ar[:], achk_r[:, :, gsl])
                nc.sync.dma_start(ai[:], achk_i[:, :, gsl])
            for ti, tn in enumerate(("q", "k")):
                for dt_ in range(HPC):
                    dsl = slice(dt_ * 128, dt_ * 128 + 128)
                    pA = psum(f"p{ti * 4 + dt_ * 2}")
                    pB = psum(f"p{ti * 4 + dt_ * 2 + 1}")
                    for h in range(HT):
                        st = (h == 0); sp = (h == HT - 1)
                        nc.tensor.matmul(pA[:], w[(tn, "r")][:, h, dsl], ar[:, h, :], start=st, stop=False)
                        nc.tensor.matmul(pA[:], w[(tn, "i")][:, h, dsl], ai[:, h, :], start=False, stop=sp)
                        nc.tensor.matmul(pB[:], w[(tn, "i")][:, h, dsl], ar[:, h, :], start=st, stop=False)
                        nc.tensor.matmul(pB[:], w[(tn, "rn")][:, h, dsl], ai[:, h, :], start=False, stop=sp)
                    rope_drain(tn, dt_, gsl, pA, pB)
            for tt in range(GW // 128):
                bk = g * (GW // 128) + tt
                tsl = slice(tt * 128, tt * 128 + 128)
                pV = psum(f"p{bk % 4}")
                for h in range(HT):
                    st = (h == 0); sp = (h == HT - 1)
                    nc.tensor.matmul(pV[:], ar[:, h, tsl], wv1[:, h, :], start=st, stop=False)
                    nc.tensor.matmul(pV[:], ai[:, h, tsl], wv2[:, h, :], start=False, stop=sp)
                for hd in range(HPC):
                    nc.vector.tensor_copy(vjoin[(hd, bk)][:, 0:256],
                                          pV[:, hd * 256:(hd + 1) * 256])

    # ======== attention per head; A2A after each head ========
    SC = float(1.0 / np.sqrt(2 * D))
    with tc.tile_pool(name="attn", bufs=1) as at, \
         tc.tile_pool(name="epool", bufs=2) as ep, \
         tc.tile_pool(name="tp", bufs=2) as tp:
        masks = at.tile([128, 4 * 512], f16, name="masks_t")
        nc.sync.dma_start(masks[:], A["masks"][:])
        pm_all = at.tile([128, 32], f32, name="pm_all")
        for hd in range(HPC):
            out_nat = {}
            for cp in ("r", "i"):
                for bq in range(T // 128):
                    out_nat[(cp, bq)] = at.tile([128, 128], f32, name=f"on{hd}{cp}{bq}",
                                                tag=f"on{hd}{cp}{bq}")
            for g in range(NG):
                etiles = {}
                for bk in range(4 * g + 4):
                    pS = psum(f"p{bk % 4}")
                    qsl = slice(g * 512, g * 512 + 512)
                    nc.tensor.matmul(pS[:], qrot[("k", hd, "r")][:, bk * 128:bk * 128 + 128],
                                     qrot[("q", hd, "r")][:, qsl], start=True, stop=False)
                    nc.tensor.matmul(pS[:], qrot[("k", hd, "i")][:, bk * 128:bk * 128 + 128],
                                     qrot[("q", hd, "i")][:, qsl], start=False, stop=True)
                    if bk >= 4 * g:
                        mc = (bk - 4 * g) * 512
                        nc.vector.tensor_tensor(pS[:], pS[:], masks[:, mc:mc + 512], A_OP.add)
                    e = ep.tile([128, 512], bf16, name=f"e{hd}{g}_{bk}", tag=f"e{bk}")
                    nc.scalar.activation(e[:], pS[:], mybir.ActivationFunctionType.Exp, scale=SC)
                    etiles[bk] = e
                for bq in range(4 * g, 4 * g + 4):
                    pO = psum(f"p{4 + bq % 4}", (128, 257))
                    col = (bq - 4 * g) * 128
                    for bk in range(bq + 1):
                        nc.tensor.matmul(pO[:], etiles[bk][:, col:col + 128], vjoin[(hd, bk)][:],
                                         start=(bk == 0), stop=(bk == bq))
                    rec = at.tile([128, 1], f32, name=f"rec{hd}{bq}", tag="rec")
                    nc.vector.reciprocal(rec[:], pO[:, 256:257])
                    for ci, cp in enumerate(("r", "i")):
                        o = out_nat[(cp, bq)]
                        nc.vector.tensor_scalar(o[:], pO[:, ci * 128:ci * 128 + 128], rec[:],
                                                None, A_OP.mult)
                        pc_ = pm_all[:, 16 * ci + bq:16 * ci + bq + 1]
                        if hd == 0:
                            nc.vector.tensor_reduce(pc_, o[:], mybir.AxisListType.X,
                                                    A_OP.max, apply_absolute_value=True)
                        else:
                            mx = at.tile([128, 1], f32, name=f"mxt{hd}{cp}{bq}", tag="mxt")
                            nc.vector.tensor_reduce(mx[:], o[:], mybir.AxisListType.X,
                                                    A_OP.max, apply_absolute_value=True)
                            nc.vector.tensor_tensor(pc_, pc_, mx[:], A_OP.max)

            # transpose this head's output and stage its A2A shard
            cont = cont1 if hd == 0 else cont2
            rows = ROWS1 if hd == 0 else ROWS2
            for ci, cp in enumerate(("r", "i")):
                oT = tp.tile([128, T], f16, name=f"oT{cp}{hd}", tag="oT")
                for bq in range(T // 128):
                    pTr = psum(f"p{bq % 2}", (128, 128))
                    nc.tensor.transpose(pTr[:], out_nat[(cp, bq)][:], ident[:])
                    nc.vector.tensor_copy(oT[:, bq * 128:bq * 128 + 128], pTr[:])
                cv = cont[:].rearrange("(s r) c -> r s c", r=rows)
                nc.sync.dma_start(cv[ci * 128:(ci + 1) * 128],
                                  oT[:].rearrange("a (s c) -> a s c", s=NC))
            if hd == HPC - 1:
                # pmax rows as f16 hi/lo: 4 vector ops, one PE transpose, 8 DMAs.
                # ptile col = s*8 + a*2 + j (a: hi_r,lo_r,hi_i,lo_i; j: token half)
                # so transpose output partitions are dest-major (plain slices).
                ptile = at.tile([128, 64], f16, name="ptile")
                pv = ptile[:].rearrange("p (s a j) -> p a s j", s=NC, a=4, j=2)
                pmv = pm_all[:].rearrange("p (ci s j) -> p ci s j", ci=2, s=NC, j=2)
                for ci in range(2):
                    nc.vector.tensor_copy(pv[:, 2 * ci], pmv[:, ci])
                    nc.vector.tensor_tensor(pv[:, 2 * ci + 1], pmv[:, ci],
                                            pv[:, 2 * ci], A_OP.subtract)
                ident16 = at.tile([128, 128], f16, name="ident16")
                nc.vector.tensor_copy(ident16[:], ident[:])
                pTm = ps.tile([64, 128], f16, name="ptm", tag="p6")
                nc.tensor.transpose(pTm[:], ptile[:], ident16[:])
                mT = at.tile([64, 128], f16, name="mT")
                nc.vector.tensor_copy(mT[:], pTm[:])
                for s in range(NC):
                    dst = cont2[s * ROWS2 + 2 * 128: s * ROWS2 + 2 * 128 + 4, :]
                    nc.sync.dma_start(dst.rearrange("a (j d) -> (a j) d", j=2),
                                      mT[8 * s:8 * s + 8, :])
            nc.gpsimd.collective_compute(
                "AllToAll", A_OP.bypass, replica_groups=[list(range(NC))],
                ins=[(cont1 if hd == 0 else cont2)[:].opt()],
                outs=[(ag1 if hd == 0 else ag2)[:].opt()])

    vj_ctx.__exit__(None, None, None)
    qk_ctx.__exit__(None, None, None)

    # ======== o-projection on my 256-token slice (Karatsuba) ========
    with tc.tile_pool(name="op", bufs=1) as op, \
         tc.tile_pool(name="wo", bufs=2) as wo, \
         tc.tile_pool(name="od", bufs=2) as od:
        # prefetch first wo chunk immediately (overlaps the A2A)
        wo_t = {}
        def load_wo(jb):
            for kind in ("i", "m", "p"):
                wt_ = wo.tile([128, HT, 512], f16, name=f"wo{kind}{jb}", tag=f"wo{kind}")
                nc.sync.dma_start(wt_[:], _chunked(A[f"wo_{kind}"][:, jb * 512:(jb + 1) * 512]))
                wo_t[(kind, jb)] = wt_
        load_wo(0)

        bounce = dram.tile([4, TS], f32, name="bounce")
        agrows2 = ag2[:].rearrange("(s r) c -> r s c", r=ROWS2)   # [260, 8, 256]
        srep = {}; invrep = {}
        for ci, cp in enumerate(("r", "i")):
            mhi = op.tile([1, NC, TS], f16, name=f"mhi{cp}", tag="mhi")
            mlo = op.tile([1, NC, TS], f16, name=f"mlo{cp}", tag="mlo")
            nc.sync.dma_start(mhi[:], agrows2[2 * 128 + 2 * ci: 2 * 128 + 2 * ci + 1, :, :])
            nc.sync.dma_start(mlo[:], agrows2[2 * 128 + 2 * ci + 1: 2 * 128 + 2 * ci + 2, :, :])
            m = op.tile([1, NC, TS], f32, name=f"m{cp}", tag="m")
            nc.vector.tensor_tensor(m[:], mhi[:], mlo[:], A_OP.add)
            g = op.tile([1, TS], f32, name=f"gmax{cp}")
            nc.vector.tensor_tensor(g[:], m[:, 0, :], m[:, 1, :], A_OP.max)
            for s in range(2, NC):
                nc.vector.tensor_tensor(g[:], g[:], m[:, s, :], A_OP.max)
            nc.vector.tensor_scalar(g[:], g[:], 1e-5, None, A_OP.max)
            rg = op.tile([1, TS], f32, name=f"rg{cp}")
            nc.vector.reciprocal(rg[:], g[:])
            nc.vector.tensor_scalar(rg[:], rg[:], 127.0, None, A_OP.mult)
            nc.sync.dma_start(bounce[ci:ci + 1, :], rg[:])
            iv = op.tile([1, TS], f32, name=f"iv{cp}")
            nc.vector.tensor_scalar(iv[:], g[:], float(1.0 / 127.0), None, A_OP.mult)
            nc.sync.dma_start(bounce[2 + ci:3 + ci, :], iv[:])
            sr = op.tile([128, TS], f32, name=f"srep{cp}")
            nc.sync.dma_start(sr[:], bounce[ci:ci + 1, :].to_broadcast((128, TS)))
            srep[cp] = sr
            irp = op.tile([128, TS], f32, name=f"invrep{cp}")
            nc.sync.dma_start(irp[:], bounce[2 + ci:3 + ci, :].to_broadcast((128, TS)))
            invrep[cp] = irp

        # quantize+dequantize A2A'd attention outputs to f16 (16 K-chunks/cp)
        xt_all = {}
        for hd, (ag, rows) in enumerate(((ag1, ROWS1), (ag2, ROWS2))):
            agv = ag[:].rearrange("(s r) c -> r s c", r=rows)
            for ci, cp in enumerate(("r", "i")):
                xt = op.tile([128, NC, TS], f16, name=f"xta{hd}{cp}")
                nc.sync.dma_start(xt[:], agv[ci * 128:(ci + 1) * 128])
                xt_all[(hd, cp)] = xt
        deq = {}
        for ci, cp in enumerate(("r", "i")):
            for s in range(NC):
                for hd in range(HPC):
                    kb = s * 2 + hd
                    xt = xt_all[(hd, cp)][:, s, :]
                    m32 = op.tile([128, TS], f32, name=f"m32{cp}{kb}", tag="m32")
                    nc.vector.tensor_tensor(m32[:], xt, srep[cp][:], A_OP.mult)
                    aq = op.tile([128, TS], f32, name=f"aq{cp}{kb}", tag="aq")
                    nc.vector.tensor_scalar(aq[:], m32[:], MAGIC, MAGIC, A_OP.add, A_OP.subtract)
                    dq = op.tile([128, TS], f16, name=f"dq{cp}{kb}")
                    nc.vector.tensor_tensor(dq[:], aq[:], invrep[cp][:], A_OP.mult)
                    deq[(cp, kb)] = dq
        ap_o = {}
        for kb in range(2 * NC):
            t = op.tile([128, TS], f16, name=f"apo{kb}")
            nc.vector.tensor_tensor(t[:], deq[("r", kb)][:], deq[("i", kb)][:], A_OP.add)
            ap_o[kb] = t

        for jb in range(4):
            if jb + 1 < 4:
                load_wo(jb + 1)
            for tt in range(2):
                tsl = slice(tt * 128, tt * 128 + 128)
                p1 = psum(f"p{tt * 3}")
                p2 = psum(f"p{tt * 3 + 1}")
                p3 = psum(f"p{tt * 3 + 2}")
                for kb in range(2 * NC):
                    st = (kb == 0); sp = (kb == 2 * NC - 1)
                    nc.tensor.matmul(p1[:], ap_o[kb][:, tsl], wo_t[("i", jb)][:, kb, :],
                                     start=st, stop=sp)
                for kb in range(2 * NC):
                    st = (kb == 0); sp = (kb == 2 * NC - 1)
                    nc.tensor.matmul(p2[:], deq[("r", kb)][:, tsl], wo_t[("m", jb)][:, kb, :],
                                     start=st, stop=sp)
                for kb in range(2 * NC):
                    st = (kb == 0); sp = (kb == 2 * NC - 1)
                    nc.tensor.matmul(p3[:], deq[("i", kb)][:, tsl], wo_t[("p", jb)][:, kb, :],
                                     start=st, stop=sp)
                t1 = od.tile([128, 512], f32, name=f"t1o{jb}{tt}", tag="t1o")
                nc.vector.tensor_copy(t1[:], p1[:])
                fr = od.tile([128, 512], f32, name=f"fr{jb}{tt}", tag="fr")
                nc.vector.tensor_tensor(fr[:], p2[:], t1[:], A_OP.add)
                nc.sync.dma_start(A["yr_part"][tsl, jb * 512:(jb + 1) * 512], fr[:])
                fi = od.tile([128, 512], f32, name=f"fi{jb}{tt}", tag="fi")
                nc.vector.tensor_tensor(fi[:], t1[:], p3[:], A_OP.subtract)
                nc.sync.dma_start(A["yi_part"][tsl, jb * 512:(jb + 1) * 512], fi[:])


_CACHE = {}

def _get_compiled():
    if "nc" not in _CACHE:
        from contextlib import ExitStack
        nc, A = build_nc()
        with tile.TileContext(nc) as tc:
            with ExitStack() as ctx:
                emit(nc, A, tc, ctx)
        nc.compile()
        _CACHE["nc"] = nc
    return _CACHE["nc"]


def _host_prep(hidden_real, hidden_imag, positions,
               Wq_r, Wq_i, Wk_r, Wk_i, Wv_r, Wv_i, Wo_r, Wo_i):
    fp16 = np.float16
    f = np.float32
    hr = np.asarray(hidden_real, f); hi = np.asarray(hidden_imag, f)

    def fakequant(x):
        s = (f(127.0) / np.maximum(np.abs(x).max(1, keepdims=True), f(1e-5))).astype(f)
        return (np.clip(np.round(x * s), -128.0, 127.0) / s).astype(f)

    a_r = np.ascontiguousarray(fakequant(hr).T.astype(fp16))
    a_i = np.ascontiguousarray(fakequant(hi).T.astype(fp16))

    inv_freq = (f(1.0) / (f(10000.0) ** (np.arange(D, dtype=f) / f(D)))).astype(f)
    freqs = np.asarray(positions, np.int32).astype(f)[:, None] * inv_freq[None, :]
    cos = np.cos(freqs).astype(f).astype(ml_dtypes.bfloat16).astype(fp16)
    sin = np.sin(freqs).astype(f).astype(ml_dtypes.bfloat16).astype(fp16)
    col = np.arange(512)[None, :]; row = np.arange(128)[:, None]
    masks = np.concatenate(
        [np.where(col >= 128 * m + row, f(0.0), f(-60000.0)) for m in range(4)],
        axis=1).astype(fp16)
    Wo_r = np.asarray(Wo_r, f); Wo_i = np.asarray(Wo_i, f)
    base = {
        "a_r": a_r, "a_i": a_i,
        "cosT": np.ascontiguousarray(cos.T), "sinT": np.ascontiguousarray(sin.T),
        "wo_i": np.ascontiguousarray(Wo_i.T.astype(fp16)),
        "wo_m": np.ascontiguousarray((Wo_r.T - Wo_i.T).astype(fp16)),
        "wo_p": np.ascontiguousarray((Wo_r.T + Wo_i.T).astype(fp16)),
        "masks": masks, "ident": np.eye(128, dtype=f),
    }
    in_maps = []
    for c in range(NC):
        sl = slice(c * DS, (c + 1) * DS)
        im = dict(base)
        for nm, Wr_, Wi_ in (("q", Wq_r, Wq_i), ("k", Wk_r, Wk_i)):
            Wr_ = np.asarray(Wr_, f)[sl]; Wi_ = np.asarray(Wi_, f)[sl]
            im[f"w{nm}_r"] = np.ascontiguousarray(Wr_.T.astype(fp16))
            im[f"w{nm}_i"] = np.ascontiguousarray(Wi_.T.astype(fp16))
            im[f"w{nm}_rn"] = np.ascontiguousarray((-Wr_.T).astype(fp16))
        vr = np.asarray(Wv_r, f)[sl].T.astype(fp16)   # [H, 256] cols: h0 dims | h1 dims
        vi = np.asarray(Wv_i, f)[sl].T.astype(fp16)
        im["wv1"] = np.ascontiguousarray(np.concatenate(
            [vr[:, 0:128], vi[:, 0:128], vr[:, 128:256], vi[:, 128:256]], axis=1))
        im["wv2"] = np.ascontiguousarray(np.concatenate(
            [vi[:, 0:128], -vr[:, 0:128], vi[:, 128:256], -vr[:, 128:256]], axis=1))
        in_maps.append(im)
    return in_maps


def kernel(**inputs):
    nc = _get_compiled()
    in_maps = _host_prep(**inputs)
    res = run_bass_kernel_spmd(nc, in_maps, list(range(NC)))
    yr = np.concatenate([res.results[c]["yr_part"] for c in range(NC)], axis=0)
    yi = np.concatenate([res.results[c]["yi_part"] for c in range(NC)], axis=0)
    return yr, yi


# revision 12
# speedup vs baseline: 1.1488x; 1.1488x over previous
"""nn_ComplexNetAttention on 8 trn2 NeuronCores.

Sharding: heads column-parallel for QKV+attention (2 heads/core),
per-head f16 AllToAll to redistribute attention output to
token-sharded layout, token-parallel o-projection (256 tokens/core).

Activations are shipped host-side fake-quantized (round(x*s)/s) in
f16 so every projection runs as long same-PSUM-bank accumulation
chains with no vector dependency (keeps the PE array at its ramped
2.4 GHz p-state). Attention e-tiles are bf16 (fp32 exponent range -
no exp overflow). O-projection uses the Karatsuba 3-multiply complex
product on re-quantized+dequantized f16 activations.
"""
import numpy as np
import ml_dtypes

import concourse.bass as bass
import concourse.bacc as bacc
import concourse.tile as tile
import concourse.mybir as mybir
from concourse.bass_utils import run_bass_kernel_spmd

f32 = mybir.dt.float32
f16 = mybir.dt.float16
bf16 = mybir.dt.bfloat16

T, H, NH, D = 2048, 2048, 16, 128
NC = 8
HPC = NH // NC          # heads per core = 2
DS = HPC * D            # d_out slice per core = 256
TS = T // NC            # tokens per core for o-proj = 256
HT = H // 128           # 16 contraction chunks
NG = 4                  # token groups of 512
GW = T // NG            # 512
ROWS1 = 2 * 128         # A2A#1 rows/src: h0_r | h0_i
ROWS2 = 2 * 128 + 4     # A2A#2 rows/src: h1_r | h1_i | maxr_hi/lo | maxi_hi/lo
MAGIC = float(2**23 + 2**22)
A_OP = mybir.AluOpType


def build_nc():
    nc = bacc.Bacc("TRN2", target_bir_lowering=False, debug=False, num_devices=NC)
    A = {}
    def inp(name, shape, dt=f16):
        A[name] = nc.dram_tensor(name, shape, dt, kind="ExternalInput").ap()
    inp("a_r", [H, T]); inp("a_i", [H, T])
    inp("cosT", [D, T]); inp("sinT", [D, T])
    for tn in ("q", "k"):
        inp(f"w{tn}_r", [H, DS]); inp(f"w{tn}_i", [H, DS]); inp(f"w{tn}_rn", [H, DS])
    inp("wv1", [H, 2 * DS]); inp("wv2", [H, 2 * DS])
    inp("wo_i", [H, H]); inp("wo_m", [H, H]); inp("wo_p", [H, H])
    inp("masks", [128, 4 * 512])
    inp("ident", [128, 128], f32)
    A["yr_part"] = nc.dram_tensor("yr_part", [TS, H], f32, kind="ExternalOutput").ap()
    A["yi_part"] = nc.dram_tensor("yi_part", [TS, H], f32, kind="ExternalOutput").ap()
    return nc, A


def _chunked(ap):
    """DRAM [H, w] -> [128, HT, w] view (partition, h-chunk, col)."""
    return ap.rearrange("(a b) c -> b a c", b=128)


def emit(nc, A, tc, ctx):
    const = ctx.enter_context(tc.tile_pool(name="const", bufs=1))
    ps = ctx.enter_context(tc.tile_pool(name="ps", bufs=1, space="PSUM"))
    dram = ctx.enter_context(tc.tile_pool(name="dram", bufs=1, space="DRAM"))

    ident = const.tile([128, 128], f32, name="ident_t")
    nc.sync.dma_start(ident[:], A["ident"][:])

    _pn = [0]
    def psum(tag, shape=(128, 512)):
        _pn[0] += 1
        return ps.tile(list(shape), f32, name=f"pt{_pn[0]}", tag=tag)

    cont1 = dram.tile([NC * ROWS1, TS], f16, name="cont1")
    ag1 = dram.tile([NC * ROWS1, TS], f16, name="ag1")
    cont2 = dram.tile([NC * ROWS2, TS], f16, name="cont2")
    ag2 = dram.tile([NC * ROWS2, TS], f16, name="ag2")

    achk_r = _chunked(A["a_r"][:])
    achk_i = _chunked(A["a_i"][:])

    qk_ctx = tc.tile_pool(name="qk", bufs=1)
    vj_ctx = tc.tile_pool(name="vj", bufs=1)
    qk_pool = qk_ctx.__enter__()
    vj_pool = vj_ctx.__enter__()
    qrot = {}
    for tn in ("q", "k"):
        for hd in range(HPC):
            for cp in ("r", "i"):
                qrot[(tn, hd, cp)] = qk_pool.tile([128, T], bf16, name=f"{tn}rot{hd}{cp}")
    vjoin = {}
    for hd in range(HPC):
        for bk in range(T // 128):
            vjoin[(hd, bk)] = vj_pool.tile([128, 257], bf16, name=f"vj{hd}_{bk}")
            nc.vector.memset(vjoin[(hd, bk)][:, 256:257], 1.0)

    # ======== phase A: Q/K/V projections + rope, one acts pass ========
    with tc.tile_pool(name="pw", bufs=1) as pw, \
         tc.tile_pool(name="acts", bufs=2) as acts_p, \
         tc.tile_pool(name="tmp", bufs=2) as tmp:
        w = {}
        for kind in ("r", "i", "rn"):
            w[("q", kind)] = pw.tile([128, HT, DS], f16, name=f"wq_{kind}")
        acts0 = {}
        for nm, src_ in (("ar", achk_r), ("ai", achk_i)):
            acts0[nm] = acts_p.tile([128, HT, GW], f16, name=f"{nm}0", tag=nm)
        for hh in range(0, HT, 4):
            for kind in ("r", "i", "rn"):
                nc.sync.dma_start(w[("q", kind)][:, hh:hh + 4, :],
                                  _chunked(A[f"wq_{kind}"][:])[:, hh:hh + 4, :])
            for nm, src_ in (("ar", achk_r), ("ai", achk_i)):
                nc.sync.dma_start(acts0[nm][:, hh:hh + 4, :], src_[:, hh:hh + 4, 0:GW])
        for kind in ("r", "i", "rn"):
            w[("k", kind)] = pw.tile([128, HT, DS], f16, name=f"wk_{kind}")
            nc.sync.dma_start(w[("k", kind)][:], _chunked(A[f"wk_{kind}"][:]))
        cosT = pw.tile([D, T], f16, name="cosT_t")
        nc.sync.dma_start(cosT[:], A["cosT"][:])
        sinT = pw.tile([D, T], f16, name="sinT_t")
        nc.sync.dma_start(sinT[:], A["sinT"][:])
        wv1 = pw.tile([128, HT, 2 * DS], f16, name="wv1_t")
        nc.sync.dma_start(wv1[:], _chunked(A["wv1"][:]))
        wv2 = pw.tile([128, HT, 2 * DS], f16, name="wv2_t")
        nc.sync.dma_start(wv2[:], _chunked(A["wv2"][:]))

        def rope_drain(tn, dt_, gsl, pA, pB):
            t1 = tmp.tile([128, GW], f16, name=f"t1{tn}{dt_}{gsl.start}", tag="t1")
            t2 = tmp.tile([128, GW], f16, name=f"t2{tn}{dt_}{gsl.start}", tag="t2")
            nc.vector.tensor_tensor(t1[:], pA[:], cosT[:, gsl], A_OP.mult)
            nc.vector.tensor_tensor(t2[:], pB[:], sinT[:, gsl], A_OP.mult)
            nc.vector.tensor_tensor(qrot[(tn, dt_, "r")][:, gsl], t1[:], t2[:], A_OP.subtract)
            t3 = tmp.tile([128, GW], f16, name=f"t3{tn}{dt_}{gsl.start}", tag="t1")
            t4 = tmp.tile([128, GW], f16, name=f"t4{tn}{dt_}{gsl.start}", tag="t2")
            nc.vector.tensor_tensor(t3[:], pB[:], cosT[:, gsl], A_OP.mult)
            nc.vector.tensor_tensor(t4[:], pA[:], sinT[:, gsl], A_OP.mult)
            nc.vector.tensor_tensor(qrot[(tn, dt_, "i")][:, gsl], t3[:], t4[:], A_OP.add)

        for g in range(NG):
            gsl = slice(g * GW, (g + 1) * GW)
            if g == 0:
                ar, ai = acts0["ar"], acts0["ai"]
            else:
                ar = acts_p.tile([128, HT, GW], f16, name=f"ar{g}", tag="ar")
                ai = acts_p.tile([128, HT, GW], f16, name=f"ai{g}", tag="ai")
                nc.sync.dma_start(ar[:], achk_r[:, :, gsl])
                nc.sync.dma_start(ai[:], achk_i[:, :, gsl])
            for ti, tn in enumerate(("q", "k")):
                for dt_ in range(HPC):
                    dsl = slice(dt_ * 128, dt_ * 128 + 128)
                    pA = psum(f"p{ti * 4 + dt_ * 2}")
                    pB = psum(f"p{ti * 4 + dt_ * 2 + 1}")
                    for h in range(HT):
                        st = (h == 0); sp = (h == HT - 1)
                        nc.tensor.matmul(pA[:], w[(tn, "r")][:, h, dsl], ar[:, h, :], start=st, stop=False)
                        nc.tensor.matmul(pA[:], w[(tn, "i")][:, h, dsl], ai[:, h, :], start=False, stop=sp)
                        nc.tensor.matmul(pB[:], w[(tn, "i")][:, h, dsl], ar[:, h, :], start=st, stop=False)
                        nc.tensor.matmul(pB[:], w[(tn, "rn")][:, h, dsl], ai[:, h, :], start=False, stop=sp)
                    rope_drain(tn, dt_, gsl, pA, pB)
            for tt in range(GW // 128):
                bk = g * (GW // 128) + tt
                tsl = slice(tt * 128, tt * 128 + 128)
                pV = psum(f"p{bk % 4}")
                for h in range(HT):
                    st = (h == 0); sp = (h == HT - 1)
                    nc.tensor.matmul(pV[:], ar[:, h, tsl], wv1[:, h, :], start=st, stop=False)
                    nc.tensor.matmul(pV[:], ai[:, h, tsl], wv2[:, h, :], start=False, stop=sp)
                for hd in range(HPC):
                    nc.vector.tensor_copy(vjoin[(hd, bk)][:, 0:256],
                                          pV[:, hd * 256:(hd + 1) * 256])

    # ======== attention per head; A2A after each head ========
    SC = float(1.0 / np.sqrt(2 * D))
    with tc.tile_pool(name="attn", bufs=1) as at, \
         tc.tile_pool(name="epool", bufs=2) as ep, \
         tc.tile_pool(name="tp", bufs=2) as tp:
        masks = at.tile([128, 4 * 512], f16, name="masks_t")
        nc.sync.dma_start(masks[:], A["masks"][:])
        pm_all = at.tile([128, 32], f32, name="pm_all")
        for hd in range(HPC):
            out_nat = {}
            for cp in ("r", "i"):
                for bq in range(T // 128):
                    out_nat[(cp, bq)] = at.tile([128, 128], f32, name=f"on{hd}{cp}{bq}",
                                                tag=f"on{hd}{cp}{bq}")
            for g in range(NG):
                etiles = {}
                for bk in range(4 * g + 4):
                    pS = psum(f"p{bk % 4}")
                    qsl = slice(g * 512, g * 512 + 512)
                    nc.tensor.matmul(pS[:], qrot[("k", hd, "r")][:, bk * 128:bk * 128 + 128],
                                     qrot[("q", hd, "r")][:, qsl], start=True, stop=False)
                    nc.tensor.matmul(pS[:], qrot[("k", hd, "i")][:, bk * 128:bk * 128 + 128],
                                     qrot[("q", hd, "i")][:, qsl], start=False, stop=True)
                    if bk >= 4 * g:
                        mc = (bk - 4 * g) * 512
                        nc.vector.tensor_tensor(pS[:], pS[:], masks[:, mc:mc + 512], A_OP.add)
                    e = ep.tile([128, 512], bf16, name=f"e{hd}{g}_{bk}", tag=f"e{bk}")
                    nc.scalar.activation(e[:], pS[:], mybir.ActivationFunctionType.Exp, scale=SC)
                    etiles[bk] = e
                for bq in range(4 * g, 4 * g + 4):
                    pO = psum(f"p{4 + bq % 4}", (128, 257))
                    col = (bq - 4 * g) * 128
                    for bk in range(bq + 1):
                        nc.tensor.matmul(pO[:], etiles[bk][:, col:col + 128], vjoin[(hd, bk)][:],
                                         start=(bk == 0), stop=(bk == bq))
                    rec = at.tile([128, 1], f32, name=f"rec{hd}{bq}", tag="rec")
                    nc.vector.reciprocal(rec[:], pO[:, 256:257])
                    for ci, cp in enumerate(("r", "i")):
                        o = out_nat[(cp, bq)]
                        nc.vector.tensor_scalar(o[:], pO[:, ci * 128:ci * 128 + 128], rec[:],
                                                None, A_OP.mult)
                        pc_ = pm_all[:, 16 * ci + bq:16 * ci + bq + 1]
                        if hd == 0:
                            nc.vector.tensor_reduce(pc_, o[:], mybir.AxisListType.X,
                                                    A_OP.max, apply_absolute_value=True)
                        else:
                            mx = at.tile([128, 1], f32, name=f"mxt{hd}{cp}{bq}", tag="mxt")
                            nc.vector.tensor_reduce(mx[:], o[:], mybir.AxisListType.X,
                                                    A_OP.max, apply_absolute_value=True)
                            nc.vector.tensor_tensor(pc_, pc_, mx[:], A_OP.max)

            # transpose this head's output and stage its A2A shard
            cont = cont1 if hd == 0 else cont2
            rows = ROWS1 if hd == 0 else ROWS2
            for ci, cp in enumerate(("r", "i")):
                oT = tp.tile([128, T], f16, name=f"oT{cp}{hd}", tag="oT")
                for bq in range(T // 128):
                    pTr = psum(f"p{bq % 2}", (128, 128))
                    nc.tensor.transpose(pTr[:], out_nat[(cp, bq)][:], ident[:])
                    nc.vector.tensor_copy(oT[:, bq * 128:bq * 128 + 128], pTr[:])
                cv = cont[:].rearrange("(s r) c -> r s c", r=rows)
                nc.sync.dma_start(cv[ci * 128:(ci + 1) * 128],
                                  oT[:].rearrange("a (s c) -> a s c", s=NC))
            if hd == HPC - 1:
                # pmax rows as f16 hi/lo: 4 vector ops, one PE transpose, 8 DMAs.
                # ptile col = s*8 + a*2 + j (a: hi_r,lo_r,hi_i,lo_i; j: token half)
                # so transpose output partitions are dest-major (plain slices).
                ptile = at.tile([128, 64], f16, name="ptile")
                pv = ptile[:].rearrange("p (s a j) -> p a s j", s=NC, a=4, j=2)
                pmv = pm_all[:].rearrange("p (ci s j) -> p ci s j", ci=2, s=NC, j=2)
                for ci in range(2):
                    nc.vector.tensor_copy(pv[:, 2 * ci], pmv[:, ci])
                    nc.vector.tensor_tensor(pv[:, 2 * ci + 1], pmv[:, ci],
                                            pv[:, 2 * ci], A_OP.subtract)
                ident16 = at.tile([128, 128], f16, name="ident16")
                nc.vector.tensor_copy(ident16[:], ident[:])
                pTm = ps.tile([64, 128], f16, name="ptm", tag="p6")
                nc.tensor.transpose(pTm[:], ptile[:], ident16[:])
                mT = at.tile([64, 128], f16, name="mT")
                nc.vector.tensor_copy(mT[:], pTm[:])
                for s in range(NC):
                    dst = cont2[s * ROWS2 + 2 * 128: s * ROWS2 + 2 * 128 + 4, :]
                    nc.sync.dma_start(dst.rearrange("a (j d) -> (a j) d", j=2),
                                      mT[8 * s:8 * s + 8, :])
            nc.gpsimd.collective_compute(
                "AllToAll", A_OP.bypass, replica_groups=[list(range(NC))],
                ins=[(cont1 if hd == 0 else cont2)[:].opt()],
                outs=[(ag1 if hd == 0 else ag2)[:].opt()])

    vj_ctx.__exit__(None, None, None)
    qk_ctx.__exit__(None, None, None)

    # ======== o-projection on my 256-token slice (Karatsuba) ========
    with tc.tile_pool(name="op", bufs=1) as op, \
         tc.tile_pool(name="wo", bufs=2) as wo, \
         tc.tile_pool(name="od", bufs=2) as od:
        # prefetch first wo chunk immediately (overlaps the A2A)
        wo_t = {}
        def load_wo(jb):
            for kind in ("i", "m", "p"):
                wt_ = wo.tile([128, HT, 512], f16, name=f"wo{kind}{jb}", tag=f"wo{kind}")
                nc.sync.dma_start(wt_[:], _chunked(A[f"wo_{kind}"][:, jb * 512:(jb + 1) * 512]))
                wo_t[(kind, jb)] = wt_
        load_wo(0)

        bounce = dram.tile([4, TS], f32, name="bounce")
        agrows2 = ag2[:].rearrange("(s r) c -> r s c", r=ROWS2)   # [260, 8, 256]
        srep = {}; invrep = {}
        for ci, cp in enumerate(("r", "i")):
            mhi = op.tile([1, NC, TS], f16, name=f"mhi{cp}", tag="mhi")
            mlo = op.tile([1, NC, TS], f16, name=f"mlo{cp}", tag="mlo")
            nc.sync.dma_start(mhi[:], agrows2[2 * 128 + 2 * ci: 2 * 128 + 2 * ci + 1, :, :])
            nc.sync.dma_start(mlo[:], agrows2[2 * 128 + 2 * ci + 1: 2 * 128 + 2 * ci + 2, :, :])
            m = op.tile([1, NC, TS], f32, name=f"m{cp}", tag="m")
            nc.vector.tensor_tensor(m[:], mhi[:], mlo[:], A_OP.add)
            g = op.tile([1, TS], f32, name=f"gmax{cp}")
            nc.vector.tensor_tensor(g[:], m[:, 0, :], m[:, 1, :], A_OP.max)
            for s in range(2, NC):
                nc.vector.tensor_tensor(g[:], g[:], m[:, s, :], A_OP.max)
            nc.vector.tensor_scalar(g[:], g[:], 1e-5, None, A_OP.max)
            rg = op.tile([1, TS], f32, name=f"rg{cp}")
            nc.vector.reciprocal(rg[:], g[:])
            nc.vector.tensor_scalar(rg[:], rg[:], 127.0, None, A_OP.mult)
            nc.sync.dma_start(bounce[ci:ci + 1, :], rg[:])
            iv = op.tile([1, TS], f32, name=f"iv{cp}")
            nc.vector.tensor_scalar(iv[:], g[:], float(1.0 / 127.0), None, A_OP.mult)
            nc.sync.dma_start(bounce[2 + ci:3 + ci, :], iv[:])
            sr = op.tile([128, TS], f32, name=f"srep{cp}")
            nc.sync.dma_start(sr[:], bounce[ci:ci + 1, :].to_broadcast((128, TS)))
            srep[cp] = sr
            irp = op.tile([128, TS], f32, name=f"invrep{cp}")
            nc.sync.dma_start(irp[:], bounce[2 + ci:3 + ci, :].to_broadcast((128, TS)))
            invrep[cp] = irp

        # quantize+dequantize A2A'd attention outputs to f16 (16 K-chunks/cp)
        xt_all = {}
        for hd, (ag, rows) in enumerate(((ag1, ROWS1), (ag2, ROWS2))):
            agv = ag[:].rearrange("(s r) c -> r s c", r=rows)
            for ci, cp in enumerate(("r", "i")):
                xt = op.tile([128, NC, TS], f16, name=f"xta{hd}{cp}")
                nc.sync.dma_start(xt[:], agv[ci * 128:(ci + 1) * 128])
                xt_all[(hd, cp)] = xt
        deq = {}
        for ci, cp in enumerate(("r", "i")):
            for s in range(NC):
                for hd in range(HPC):
                    kb = s * 2 + hd
                    xt = xt_all[(hd, cp)][:, s, :]
                    m32 = op.tile([128, TS], f32, name=f"m32{cp}{kb}", tag="m32")
                    nc.vector.tensor_tensor(m32[:], xt, srep[cp][:], A_OP.mult)
                    aq = op.tile([128, TS], f32, name=f"aq{cp}{kb}", tag="aq")
                    nc.vector.tensor_scalar(aq[:], m32[:], MAGIC, MAGIC, A_OP.add, A_OP.subtract)
                    dq = op.tile([128, TS], f16, name=f"dq{cp}{kb}")
                    nc.vector.tensor_tensor(dq[:], aq[:], invrep[cp][:], A_OP.mult)
                    deq[(cp, kb)] = dq
        ap_o = {}
        for kb in range(2 * NC):
            t = op.tile([128, TS], f16, name=f"apo{kb}")
            nc.vector.tensor_tensor(t[:], deq[("r", kb)][:], deq[("i", kb)][:], A_OP.add)
            ap_o[kb] = t

        for jb in range(4):
            if jb + 1 < 4:
                load_wo(jb + 1)
            for tt in range(2):
                tsl = slice(tt * 128, tt * 128 + 128)
                p1 = psum(f"p{tt * 3}")
                p2 = psum(f"p{tt * 3 + 1}")
                p3 = psum(f"p{tt * 3 + 2}")
                for kb in range(2 * NC):
                    st = (kb == 0); sp = (kb == 2 * NC - 1)
                    nc.tensor.matmul(p1[:], ap_o[kb][:, tsl], wo_t[("i", jb)][:, kb, :],
                                     start=st, stop=sp)
                for kb in range(2 * NC):
                    st = (kb == 0); sp = (kb == 2 * NC - 1)
                    nc.tensor.matmul(p2[:], deq[("r", kb)][:, tsl], wo_t[("m", jb)][:, kb, :],
                                     start=st, stop=sp)
                for kb in range(2 * NC):
                    st = (kb == 0); sp = (kb == 2 * NC - 1)
                    nc.tensor.matmul(p3[:], deq[("i", kb)][:, tsl], wo_t[("p", jb)][:, kb, :],
                                     start=st, stop=sp)
                t1 = od.tile([128, 512], f32, name=f"t1o{jb}{tt}", tag="t1o")
                nc.vector.tensor_copy(t1[:], p1[:])
                fr = od.tile([128, 512], f32, name=f"fr{jb}{tt}", tag="fr")
                nc.vector.tensor_tensor(fr[:], p2[:], t1[:], A_OP.add)
                nc.sync.dma_start(A["yr_part"][tsl, jb * 512:(jb + 1) * 512], fr[:])
                fi = od.tile([128, 512], f32, name=f"fi{jb}{tt}", tag="fi")
                nc.vector.tensor_tensor(fi[:], t1[:], p3[:], A_OP.subtract)
                nc.sync.dma_start(A["yi_part"][tsl, jb * 512:(jb + 1) * 512], fi[:])


_CACHE = {}

def _get_compiled():
    if "nc" not in _CACHE:
        from contextlib import ExitStack
        nc, A = build_nc()
        with tile.TileContext(nc) as tc:
            with ExitStack() as ctx:
                emit(nc, A, tc, ctx)
        nc.compile()
        _CACHE["nc"] = nc
    return _CACHE["nc"]


def _host_prep(hidden_real, hidden_imag, positions,
               Wq_r, Wq_i, Wk_r, Wk_i, Wv_r, Wv_i, Wo_r, Wo_i):
    fp16 = np.float16
    f = np.float32
    hr = np.asarray(hidden_real, f); hi = np.asarray(hidden_imag, f)

    def fakequant(x):
        s = (f(127.0) / np.maximum(np.abs(x).max(1, keepdims=True), f(1e-5))).astype(f)
        return (np.clip(np.round(x * s), -128.0, 127.0) / s).astype(f)

    a_r = np.ascontiguousarray(fakequant(hr).T.astype(fp16))
    a_i = np.ascontiguousarray(fakequant(hi).T.astype(fp16))

    inv_freq = (f(1.0) / (f(10000.0) ** (np.arange(D, dtype=f) / f(D)))).astype(f)
    freqs = np.asarray(positions, np.int32).astype(f)[:, None] * inv_freq[None, :]
    cos = np.cos(freqs).astype(f).astype(ml_dtypes.bfloat16).astype(fp16)
    sin = np.sin(freqs).astype(f).astype(ml_dtypes.bfloat16).astype(fp16)
    col = np.arange(512)[None, :]; row = np.arange(128)[:, None]
    masks = np.concatenate(
        [np.where(col >= 128 * m + row, f(0.0), f(-60000.0)) for m in range(4)],
        axis=1).astype(fp16)
    Wo_r = np.asarray(Wo_r, f); Wo_i = np.asarray(Wo_i, f)
    base = {
        "a_r": a_r, "a_i": a_i,
        "cosT": np.ascontiguousarray(cos.T), "sinT": np.ascontiguousarray(sin.T),
        "wo_i": np.ascontiguousarray(Wo_i.T.astype(fp16)),
        "wo_m": np.ascontiguousarray((Wo_r.T - Wo_i.T).astype(fp16)),
        "wo_p": np.ascontiguousarray((Wo_r.T + Wo_i.T).astype(fp16)),
        "masks": masks, "ident": np.eye(128, dtype=f),
    }
    in_maps = []
    for c in range(NC):
        sl = slice(c * DS, (c + 1) * DS)
        im = dict(base)
        for nm, Wr_, Wi_ in (("q", Wq_r, Wq_i), ("k", Wk_r, Wk_i)):
            Wr_ = np.asarray(Wr_, f)[sl]; Wi_ = np.asarray(Wi_, f)[sl]
            im[f"w{nm}_r"] = np.ascontiguousarray(Wr_.T.astype(fp16))
            im[f"w{nm}_i"] = np.ascontiguousarray(Wi_.T.astype(fp16))
            im[f"w{nm}_rn"] = np.ascontiguousarray((-Wr_.T).astype(fp16))
        vr = np.asarray(Wv_r, f)[sl].T.astype(fp16)   # [H, 256] cols: h0 dims | h1 dims
        vi = np.asarray(Wv_i, f)[sl].T.astype(fp16)
        im["wv1"] = np.ascontiguousarray(np.concatenate(
            [vr[:, 0:128], vi[:, 0:128], vr[:, 128:256], vi[:, 128:256]], axis=1))
        im["wv2"] = np.ascontiguousarray(np.concatenate(
            [vi[:, 0:128], -vr[:, 0:128], vi[:, 128:256], -vr[:, 128:256]], axis=1))
        in_maps.append(im)
    return in_maps


def kernel(**inputs):
    nc = _get_compiled()
    in_maps = _host_prep(**inputs)
    res = run_bass_kernel_spmd(nc, in_maps, list(range(NC)))
    yr = np.concatenate([res.results[c]["yr_part"] for c in range(NC)], axis=0)
    yi = np.concatenate([res.results[c]["yi_part"] for c in range(NC)], axis=0)
    return yr, yi
